# revision 21
# baseline (speedup 1.0000x reference)
"""GatedAttentionUnit Trainium2 kernel.

Shapes (hardcoded): B=4, S=2048, D=768, I=1536, HEAD_DIM=128.

Sharding: 8 cores = 4 batches x 2 halves of the inner dim I.
Each core computes, for its batch b and I-half h:
    v_h    = silu(x_b @ v_W[:, h])          (S, 768)   [key side, full S]
    gateT_h= silu(x_b @ g_W[:, h]).T        (768, S)
    baseT  = silu(x_b @ in_W + in_b).T      (128, S)
    qT/kT  = affine(baseT)                  (128, S)   [q pre-scaled by 1/sqrt(I)]
    attnT  = relu(kT.T q + bias_masked)^2   (S, S) lower-triangular, tiled
    oT_h   = v_h.T-contract attnT           (768, S)
    tT_h   = oT_h * gateT_h
    part   = (tT_h.T @ out_W[h])            (S, 768)  fp32 partial
Host: out[b] = part[2b] + part[2b+1] + out_b.

All matmul operands fp16, PSUM fp32. Bias+causal-mask is host-precomputed as
16 Toeplitz tiles (128x512) indexed by block offset.

NOTE on sync: trn2 instructions support at most ONE semaphore wait. The tiny
"absorber" ops (1-element copies / 1x8 matmuls) advance each engine's vector
clock over one semaphore at a time so no real instruction ever needs two
fresh waits.
"""

import numpy as np
from contextlib import ExitStack

import concourse.bass as bass
from concourse import bacc
import concourse.tile as tile
import concourse.mybir as mybir
from concourse.bass_utils import run_bass_kernel_spmd

FP16 = mybir.dt.float16
FP32 = mybir.dt.float32
AF = mybir.ActivationFunctionType
ALU = mybir.AluOpType

B, S, D, I = 4, 2048, 768, 1536
HD = 128
IH = I // 2           # 768 per-core I half
ND = D // 128         # 6 contraction blocks over D
NIB = IH // 128       # 6 blocks over I half
NKT = S // 128        # 16 key tiles
NQT = S // 128        # 16 query tiles (final matmul)
QB = 512              # query block width in phase B
NQB = S // QB         # 4
NBT = 16              # distinct Toeplitz bias tiles

NUM_BUCKETS = 32
MAX_DISTANCE = 128
MASK_VAL = -30000.0   # -inf substitute; relu clamps to 0


def _bias_by_distance(rel_emb):
    """f(d) for d in 0..S-1: rel_emb[bucket(d)] * sqrt(HD), T5 causal bucketing.

    Mirrors the reference's jax ops exactly (fp32 log boundary cases differ
    between numpy and XLA, shifting ~2% of buckets by one).
    """
    import jax.numpy as jnp
    n = jnp.arange(S)
    max_exact = NUM_BUCKETS // 2
    n_safe = jnp.maximum(n, 1).astype(jnp.float32)
    val_large = max_exact + (
        jnp.log(n_safe / max_exact) / np.log(MAX_DISTANCE / max_exact)
        * (NUM_BUCKETS - max_exact)
    ).astype(jnp.int32)
    val_large = jnp.minimum(val_large, NUM_BUCKETS - 1)
    bucket = np.asarray(jnp.where(n < max_exact, n, val_large))
    return (rel_emb[bucket, 0] * np.sqrt(np.float32(HD))).astype(np.float32)


def _build_bias_tiles(rel_emb):
    """(128, 16, 512) fp16: tile t holds scores-bias for block offset (t-3)*128.

    scoresT tile layout: [key 128 partitions, query 512 free]. Entry (r, c) of
    tile t covers distance dd = (t-3)*128 + c - r; dd < 0 is causal-masked.
    """
    f = _bias_by_distance(rel_emb)
    t = np.arange(NBT)[:, None, None]
    r = np.arange(128)[None, :, None]
    c = np.arange(QB)[None, None, :]
    dd = (t - 3) * 128 + c - r
    tiles = np.where(dd >= 0, f[np.clip(dd, 0, S - 1)], np.float32(MASK_VAL))
    return np.ascontiguousarray(tiles.transpose(1, 0, 2).astype(np.float16))


_PROGRAM = None
_TRACE = False          # set True (e.g. from test.py) to capture NTFF profile
_LAST_RESULT = None     # BassKernelResults of the most recent run


def _build_program(with_vb):
    nc = bacc.Bacc()
    d_xT = nc.declare_dram_parameter("xT", [128, ND, S], FP16, isOutput=False)
    d_vW = nc.declare_dram_parameter("vW", [128, ND, IH], FP16, isOutput=False)
    d_gW = nc.declare_dram_parameter("gW", [128, ND, IH], FP16, isOutput=False)
    d_inW = nc.declare_dram_parameter("inW", [128, ND, HD], FP16, isOutput=False)
    d_outW = nc.declare_dram_parameter("outW", [128, NIB, D], FP16, isOutput=False)
    d_biasT = nc.declare_dram_parameter("biasT", [128, NBT, QB], FP16, isOutput=False)
    d_scal = nc.declare_dram_parameter("scal", [128, 16], FP32, isOutput=False)
    if with_vb:
        d_vb = nc.declare_dram_parameter("vb", [1, IH], FP16, isOutput=False)
    d_out = nc.declare_dram_parameter("out", [S, D], FP16, isOutput=True)

    with tile.TileContext(nc) as tc, ExitStack() as ctx:
        const = ctx.enter_context(tc.tile_pool(name="const", bufs=1))
        work = ctx.enter_context(tc.tile_pool(name="work", bufs=3))

        xT = const.tile([128, ND, S], FP16)
        vW = const.tile([128, ND, IH], FP16)
        gW = const.tile([128, ND, IH], FP16)
        inW = const.tile([128, ND, HD], FP16)
        outW = const.tile([128, NIB, D], FP16)
        biasT = const.tile([128, NBT, QB], FP16)
        scal = const.tile([128, 16], FP32)
        nc.sync.dma_start(out=xT[:], in_=d_xT[:])
        nc.sync.dma_start(out=vW[:], in_=d_vW[:])
        nc.sync.dma_start(out=gW[:], in_=d_gW[:])
        nc.sync.dma_start(out=inW[:], in_=d_inW[:])
        nc.sync.dma_start(out=outW[:], in_=d_outW[:])
        nc.sync.dma_start(out=biasT[:], in_=d_biasT[:])
        nc.sync.dma_start(out=scal[:], in_=d_scal[:])
        if with_vb:
            vb = const.tile([1, IH], FP16)
            nc.sync.dma_start(out=vb[:], in_=d_vb[:])
            ones1 = const.tile([1, 128], FP16)
            nc.vector.memset(ones1[:], 1.0)

        v_s = const.tile([128, NKT, IH], FP16)    # [k_part, kb, i]
        gT_s = const.tile([128, NIB, S], FP16)    # [i_part, ib, q]
        qT_s = const.tile([128, S], FP16)         # [hd, q]
        kT_s = const.tile([128, S], FP16)         # [hd, k]
        tT_s = const.tile([128, NIB, S], FP16)    # [i_part, ib, q]
        out_s = const.tile([128, NQT, D], FP16)   # [q_part, qt, d] staging

        # ---- Warmup absorbers: one new semaphore per instruction ----
        # Single PSUM pool for the whole program: tag "big" (2 bufs) is shared
        # by every phase; o0..o5 hold phase-B accumulators. 8 banks total, no
        # pool-boundary release edges (those add PE-self waits walrus rejects).
        ps = ctx.enter_context(tc.tile_pool(name="ps", bufs=2, space="PSUM"))

        # ---- Phase A1: baseT -> qT, kT ----
        for qb in range(NQB):
            bp = ps.tile([128, QB], FP32, tag="big", name="bp")
            for d in range(ND):
                nc.tensor.matmul(
                    bp[:], inW[:, d, :], xT[:, d, qb * QB:(qb + 1) * QB],
                    start=(d == 0), stop=(d == ND - 1))
            base_f = work.tile([128, QB], FP32, tag="base", bufs=4)
            nc.scalar.activation(base_f[:], bp[:], AF.Silu, bias=scal[:, 0:1])
            nc.vector.tensor_scalar(
                out=qT_s[:, qb * QB:(qb + 1) * QB], in0=base_f[:],
                scalar1=scal[:, 1:2], scalar2=scal[:, 2:3],
                op0=ALU.mult, op1=ALU.add)
            nc.vector.tensor_scalar(
                out=kT_s[:, qb * QB:(qb + 1) * QB], in0=base_f[:],
                scalar1=scal[:, 3:4], scalar2=scal[:, 4:5],
                op0=ALU.mult, op1=ALU.add)

        # ---- Phase A2: v (rows, IH) ----
        for rt in range(NKT):
            if rt % 2 == 0:
                p1 = ps.tile([128, 512], FP32, tag="big", name="p1")
                p2 = ps.tile([128, 256], FP32, tag="big", name="p2")
            else:
                p1 = ps.tile([128, 512], FP32, tag="o0", name="p1b", bufs=1)
                p2 = ps.tile([128, 256], FP32, tag="o1", name="p2b", bufs=1)
            for d in range(ND):
                lhsT = xT[:, d, rt * 128:(rt + 1) * 128]
                nc.tensor.matmul(p1[:], lhsT, vW[:, d, 0:512],
                                 start=(d == 0), stop=(d == ND - 1 and not with_vb))
                nc.tensor.matmul(p2[:], lhsT, vW[:, d, 512:768],
                                 start=(d == 0), stop=(d == ND - 1 and not with_vb))
            if with_vb:
                nc.tensor.matmul(p1[:], ones1[:], vb[:, 0:512],
                                 start=False, stop=True)
                nc.tensor.matmul(p2[:], ones1[:], vb[:, 512:768],
                                 start=False, stop=True)
            nc.scalar.activation(v_s[:, rt, 0:512], p1[:], AF.Silu)
            nc.scalar.activation(v_s[:, rt, 512:768], p2[:], AF.Silu)

        # ---- Phase A3: gateT (IH, S) ----
        for ib in range(NIB):
            for qb in range(NQB):
                gp = ps.tile([128, QB], FP32, tag="big", name="gp")
                for d in range(ND):
                    nc.tensor.matmul(
                        gp[:], gW[:, d, ib * 128:(ib + 1) * 128],
                        xT[:, d, qb * QB:(qb + 1) * QB],
                        start=(d == 0), stop=(d == ND - 1))
                nc.scalar.activation(gT_s[:, ib, qb * QB:(qb + 1) * QB],
                                     gp[:], AF.Silu, bias=scal[:, 5 + ib:6 + ib])

        # ---- Phase B: scores -> relu^2 -> oT -> tT ----
        for qb in range(NQB):
            ops = [ps.tile([128, QB], FP32, tag=f"o{ib}", name=f"ops{ib}", bufs=1)
                   for ib in range(NIB)]
            nkb = 4 * qb + 4
            sps = [None] * nkb
            abs_ = [None] * nkb

            def emit_scores(kb, qb=qb):
                sp = ps.tile([128, QB], FP32, tag="big", name="sp")
                nc.tensor.matmul(sp[:], kT_s[:, kb * 128:(kb + 1) * 128],
                                 qT_s[:, qb * QB:(qb + 1) * QB],
                                 start=True, stop=True)
                return sp

            sps[0] = emit_scores(0)
            for kb in range(nkb):
                # software pipeline: next scores before this kb's oT matmuls
                if kb + 1 < nkb:
                    sps[kb + 1] = emit_scores(kb + 1)
                sp = sps[kb]
                tix = 4 * qb - kb + 3
                sb = work.tile([128, QB], FP32, tag="sb", bufs=3)
                nc.vector.tensor_tensor(out=sb[:], in0=sp[:],
                                        in1=biasT[:, tix, :], op=ALU.add)
                rb = work.tile([128, QB], FP32, tag="rb", bufs=3)
                nc.vector.tensor_scalar_max(rb[:], sb[:], 0.0)
                ab = work.tile([128, QB], FP16, tag="ab", bufs=4)
                nc.vector.tensor_tensor(out=ab[:], in0=rb[:], in1=rb[:],
                                        op=ALU.mult)
                for ib in range(NIB):
                    nc.tensor.matmul(ops[ib][:],
                                     v_s[:, kb, ib * 128:(ib + 1) * 128], ab[:],
                                     start=(kb == 0), stop=(kb == nkb - 1))
            for ib in range(NIB):
                nc.vector.tensor_tensor(
                    out=tT_s[:, ib, qb * QB:(qb + 1) * QB], in0=ops[ib][:],
                    in1=gT_s[:, ib, qb * QB:(qb + 1) * QB], op=ALU.mult)

        # ---- Phase C: out = tT.T @ out_W ----
        for qt in range(NQT):
            # alternate psum pairs: ("big","big") and retired B banks (o0,o1)
            if qt % 2 == 0:
                f1 = ps.tile([128, 512], FP32, tag="big", name="f1")
                f2 = ps.tile([128, 256], FP32, tag="big", name="f2")
            else:
                f1 = ps.tile([128, 512], FP32, tag="o0", name="f1b", bufs=1)
                f2 = ps.tile([128, 256], FP32, tag="o1", name="f2b", bufs=1)
            for ib in range(NIB):
                lhsT = tT_s[:, ib, qt * 128:(qt + 1) * 128]
                nc.tensor.matmul(f1[:], lhsT, outW[:, ib, 0:512],
                                 start=(ib == 0), stop=(ib == NIB - 1))
                nc.tensor.matmul(f2[:], lhsT, outW[:, ib, 512:768],
                                 start=(ib == 0), stop=(ib == NIB - 1))
            nc.scalar.copy(out_s[:, qt, 0:512], f1[:])
            nc.scalar.copy(out_s[:, qt, 512:768], f2[:])
            nc.sync.dma_start(out=d_out[qt * 128:(qt + 1) * 128, :],
                              in_=out_s[:, qt, :])

    nc.compile()
    return nc


def _get_program(with_vb):
    global _PROGRAM
    if _PROGRAM is None or _PROGRAM[1] != with_vb:
        _PROGRAM = (_build_program(with_vb), with_vb)
    return _PROGRAM[0]


def _pack_dblk(w):
    """(D, N) -> (128, D//128, N): w[d*128+p, n] -> out[p, d, n], fp16."""
    Dd, N = w.shape
    return np.ascontiguousarray(
        w.reshape(Dd // 128, 128, N).transpose(1, 0, 2).astype(np.float16))


def kernel(**inputs):
    x = np.asarray(inputs["x"], np.float32)
    v_W = np.asarray(inputs["v_W"], np.float32)
    v_b = np.asarray(inputs["v_b"], np.float32)
    g_W = np.asarray(inputs["g_W"], np.float32)
    g_b = np.asarray(inputs["g_b"], np.float32)
    in_W = np.asarray(inputs["in_W"], np.float32)
    in_b = np.asarray(inputs["in_b"], np.float32)
    q_gamma = np.asarray(inputs["q_gamma"], np.float32)
    q_beta = np.asarray(inputs["q_beta"], np.float32)
    k_gamma = np.asarray(inputs["k_gamma"], np.float32)
    k_beta = np.asarray(inputs["k_beta"], np.float32)
    out_W = np.asarray(inputs["out_W"], np.float32)
    out_b = np.asarray(inputs["out_b"], np.float32)
    rel_emb = np.asarray(inputs["rel_emb"], np.float32)

    with_vb = bool(np.any(v_b != 0))
    nc = _get_program(with_vb)

    biasT_h = _build_bias_tiles(rel_emb)
    inW_h = _pack_dblk(in_W)
    scale = np.float32(1.0 / np.sqrt(I))

    in_maps = []
    for c in range(8):
        b, h = c // 2, c % 2
        sl = slice(h * IH, (h + 1) * IH)
        xT_h = np.ascontiguousarray(
            x[b].T.reshape(ND, 128, S).transpose(1, 0, 2).astype(np.float16))
        scal_h = np.zeros((128, 16), np.float32)
        scal_h[:, 0] = in_b
        scal_h[:, 1] = q_gamma * scale
        scal_h[:, 2] = q_beta * scale
        scal_h[:, 3] = k_gamma
        scal_h[:, 4] = k_beta
        gb_h = g_b[sl]
        for ib in range(NIB):
            scal_h[:, 5 + ib] = gb_h[ib * 128:(ib + 1) * 128]
        m = {
            "xT": xT_h,
            "vW": _pack_dblk(v_W[:, sl]),
            "gW": _pack_dblk(g_W[:, sl]),
            "inW": inW_h,
            "outW": _pack_dblk(out_W[sl, :]),
            "biasT": biasT_h,
            "scal": scal_h,
        }
        if with_vb:
            m["vb"] = v_b[sl].reshape(1, IH).astype(np.float16)
        in_maps.append(m)

    global _LAST_RESULT
    res = run_bass_kernel_spmd(nc, in_maps, core_ids=list(range(8)),
                               trace=_TRACE)
    _LAST_RESULT = res
    out = np.empty((B, S, D), np.float32)
    for b in range(B):
        out[b] = (res.results[2 * b]["out"].astype(np.float32)
                  + res.results[2 * b + 1]["out"].astype(np.float32))
    out += out_b
    return out
